# revision 44
# baseline (speedup 1.0000x reference)
"""STSPBlock Trainium2 kernel (v2 — PE-dense pipelined schedule).

Structure (per core, batch-sharded B=16 -> 8 cores x B=2):
  partitions p = b*64 + channel for all activation tensors.

  - conv0+bn+LIF-input-scale folded into one K=110 im2col matmul in
    BF16: x is split HOST-side into x1 = bf16(x) and x2 = bf16(x - x1)
    and conv weights into W1/W2 likewise; rows = 2 bias + 36 (W1 vs x1)
    + 36 (W1 vs x2) + 36 (W2 vs x1), so the conv is exact to ~2^-16.
    x1/x2 are staged by the HOST as flat zero-padded arrays (66-wide
    rows, 128-elem head/tail pad), so the kernel DMAs taps straight
    from the input with no DRAM->DRAM pre-pass.  The LIF state add
    (1-c0)*v rides the same PSUM group via a scaled-identity fp32r
    matmul.

  - Spikes s' = Sign(u - 1) in {-1,+1} on ACT ("sign encoding",
    s = (s'+1)/2); every consumer is linear in s so the affine fix is
    folded host-side.  Spike tiles are BF16 (exact for +-1), which
    enables 2x/4x DVE modes on the pooling / y-combination ops.

  - LIF reset needs no spike tile: v' = (u < thr) * u is one DVE
    scalar_tensor_tensor reading PSUM twice, so resets never wait on
    the ACT engine.

  - Node convs use STATIC block-diag weights; the per-step alpha scale
    is handled by dividing the whole PSUM through by a = alpha*cn:
    psn' = u/a = conv_taps(o0) + [cn*bias]*(1/a) + (1-cn)*(a_prev/a)*vt
    where vt = v/a_prev is the stored (scaled) LIF state.  1/a and the
    ratio a_prev/a are tiny [2,4] DVE ops; they are broadcast to all
    128 partitions via one small matmul (gbc), giving per-partition
    Sign biases (-1/a), reset thresholds (+1/a) and identity scales.
    This removes the per-step [128,1152] weight-scaling ACT copies.

  - 2x2 avgpool: vertical spike pairs on GPSIMD (bf16), horizontal
    pairs + padded-o0 assembly on GPSIMD; f0 row-sums ride the conv0
    Sign's accum_out (free-dim reduce on ACT).

  - y = affine combination of sign spikes: first term on GPSIMD,
    remaining three as BF16 4x stt ops on DVE; y is written to DRAM in
    BF16 and converted to fp32 on the host (~2^-9 relative, well
    inside the 2e-2 gate).

  - Schedule: each block emits conv(t) chunks interleaved with the
    graph-block tiny matmuls of step t-1, then the node units of t-1
    with taps/finish staggered, so the in-order PE always has ready
    work and stays at full clock.  node-tail trace updates of t-2 run
    at the head of the block.

All bn/LIF/sigmoid parameter folding is done host-side from the actual
input values at call time, so the kernel is fully general.
"""

import numpy as np
import ml_dtypes

import concourse.bass as bass
import concourse.bacc as bacc
import concourse.mybir as mybir
from concourse.tile import TileContext
from concourse.bass_utils import run_bass_kernel_spmd

FP = mybir.dt.float32
FPR = mybir.dt.float32r
BF = mybir.dt.bfloat16
Alu = mybir.AluOpType
Act = mybir.ActivationFunctionType

T, BFULL, CIN, H, W = 8, 16, 2, 64, 64
CO, NN, HEADS = 64, 4, 4
HP, WP = 32, 32
BC = 2                    # batch per core
NCORES = 8
EPS = 1e-5
DECAY = 0.6
HD = CO // HEADS          # 16
PLT = 16896               # per-timestep plane group (2b x 2ci x 64 x 66)
XLEN = T * PLT + 256      # flat padded x length

ACT_SET_NLE = None  # index of the Sign+Exp+Ln+Copy ACT table set

BF16 = ml_dtypes.bfloat16


def _bf16_split(a):
    hi = np.asarray(a, np.float32).astype(BF16)
    lo = (np.asarray(a, np.float32) - hi.astype(np.float32)).astype(BF16)
    return hi, lo


# ----------------------------------------------------------------- host consts
def _host_consts(conv0_w, bn0_g, bn0_b, bn0_m, bn0_v, lif0_w,
                 convs_w, bns_g, bns_b, bns_m, bns_v, lifs_w,
                 ft_w, ft_b, gat_w, gat_a, out_weights):
    f32 = np.float32
    sig = lambda z: 1.0 / (1.0 + np.exp(-z.astype(np.float64)))
    c0 = f32(sig(lif0_w))
    cn = sig(lifs_w).astype(f32)          # [3]

    s0c = (bn0_g / np.sqrt(bn0_v + EPS)).astype(f32)
    bias0 = ((bn0_b - bn0_m * s0c) * c0).astype(f32)
    W0f = (conv0_w * s0c[:, None, None, None] * c0).astype(f32)  # [64,2,3,3]

    # BF16 split of weights and bias: products W1*x1 + W1*x2 + W2*x1
    # reproduce W*x to ~2^-16 relative.
    W1, W2 = _bf16_split(W0f)
    b1, b2 = _bf16_split(bias0)
    W1f, W2f = W1.astype(f32), W2.astype(f32)
    b1f, b2f = b1.astype(f32), b2.astype(f32)

    # w0bd [110, 3*128] BF16: rows 0/1 = bias hi/lo (ones taps); rows
    # 2-37 = W1 (vs x1), 38-73 = W1 (vs x2), 74-109 = W2 (vs x1).
    # col m = v*128 + b*64 + co.  x rows are host-padded to 66 wide with
    # zero pad columns, so horizontal tap overflow reads zeros; only
    # vertical overflow needs fixing: variant 1 subtracts the dy=0
    # taps' garbage at h=0, variant 2 the dy=2 taps' at h=63.
    w0bd = np.zeros((110, 3 * 128), f32)
    w0bd[0, 0:64] = b1f
    w0bd[0, 64:128] = b1f
    w0bd[1, 0:64] = b2f
    w0bd[1, 64:128] = b2f

    def put(v, dy, dx, sgn):
        for b in range(2):
            for ci in range(2):
                p = dy * 12 + dx * 4 + b * 2 + ci
                c0_, c1_ = v * 128 + b * 64, v * 128 + (b + 1) * 64
                w0bd[2 + p, c0_:c1_] = sgn * W1f[:, ci, dy, dx]
                w0bd[38 + p, c0_:c1_] = sgn * W1f[:, ci, dy, dx]
                w0bd[74 + p, c0_:c1_] = sgn * W2f[:, ci, dy, dx]

    for dy in range(3):
        for dx in range(3):
            put(0, dy, dx, 1.0)
    for dx in range(3):
        put(1, 0, dx, -1.0)
        put(2, 2, dx, -1.0)

    i0 = ((1.0 - c0) * np.eye(128)).astype(f32)

    sncol = (bns_g / np.sqrt(bns_v + EPS)).astype(f32)            # [3,64]
    biasn_raw = (bns_b - bns_m * sncol).astype(f32)               # [3,64]
    # 0.25 = avgpool fold; extra 0.5 = sign-encoding decode s=(s'+1)/2
    Wf = (convs_w * sncol[:, :, None, None, None] * 0.25).astype(f32)
    Wh = (Wf * 0.5).astype(f32)

    # wnodc [64, 27*64]: COMPACT per (node, tap) lhsT blocks [ci, co];
    # expanded on-chip into the block-diag [128, 27*128] via two
    # pattern-DMAs (both batches share the same block).  STATIC —
    # alpha handled by dividing the PSUM through by a.
    wnodc = np.zeros((64, 27 * 64), f32)
    for n in range(3):
        for dy in range(3):
            for dx in range(3):
                k = n * 9 + dy * 3 + dx
                wnodc[:, k * 64:(k + 1) * 64] = Wh[n, :, :, dy, dx].T

    in3 = np.stack([(1.0 - cn[n]) * np.eye(128) for n in range(3)]).astype(f32)

    # bap [4, 3*128]: rows 0/1 = cn*bias_n at batch-b partitions
    # (pair with per-batch 1/a rhs rows); rows 2/3 = sign-encoding
    # uniform term 2*sum_taps(Wf) at batch-b partitions (pair with
    # static ones rhs rows).
    tapsum = Wf.sum(axis=(2, 3, 4))            # [3, 64]
    bap = np.zeros((4, 3 * 128), f32)
    for n in range(3):
        for b in range(2):
            bap[b, n * 128 + b * 64:n * 128 + (b + 1) * 64] = cn[n] * biasn_raw[n]
            bap[2 + b, n * 128 + b * 64:n * 128 + (b + 1) * 64] = 2.0 * tapsum[n]

    def bd(m):  # block-diag [128,128] of m.T twice ([co,ci] -> lhsT)
        z = np.zeros((128, 128), f32)
        z[0:64, 0:64] = m.T
        z[64:128, 64:128] = m.T
        return z

    # feat transform: f04 = 0.4*relu(ftw @ mean + ftb), sign-decode and
    # the 0.4 trace factor folded:  mean0 = (0.125/1024)*S0sum + 0.5,
    # meann = (0.5/1024)*Snsum + 0.5.
    ftmm = np.stack([bd(ft_w * (0.4 * 0.125 / 1024.0)),
                     bd(ft_w * (0.4 * 0.5 / 1024.0))])
    ftb_f = (0.4 * (ft_b + 0.5 * ft_w.sum(axis=1))).astype(f32)
    ftb2 = np.tile(ftb_f, 2).reshape(128, 1).astype(f32)
    gwbd = bd(gat_w).astype(f32)

    # ga1/ga2 [128, 8]: in p=(b, c') c'=h*16+d ; out m = b*4+h
    ga1 = np.zeros((128, 8), f32)
    ga2 = np.zeros((128, 8), f32)
    for b in range(2):
        for h in range(HEADS):
            for d in range(HD):
                ga1[b * 64 + h * 16 + d, b * 4 + h] = gat_a[h, d]
                ga2[b * 64 + h * 16 + d, b * 4 + h] = gat_a[h, HD + d]

    # ghbd [8,2]: p=(b,h) -> col b ; carries 0.5(sym)*0.25(mean h)/0.01(temp)
    ghbd = np.zeros((8, 2), f32)
    for b in range(2):
        ghbd[b * 4:(b + 1) * 4, b] = 12.5

    gbc = np.zeros((2, 128), f32)
    gbc[0, 0:64] = 1.0
    gbc[1, 64:128] = 1.0

    # cnrow4 [2,4]: col 0 unused (node 0 has no conv), cols 1-3 = cn
    cnrow4 = np.zeros((2, 4), f32)
    cnrow4[:, 1:4] = cn[None, :]

    def cols(stk):  # [k,128,128] -> [128, k*128]
        return np.ascontiguousarray(
            np.transpose(stk, (1, 0, 2)).reshape(128, -1))

    return dict(w0bd=w0bd, i0=i0, wnodc=wnodc,
                in3=cols(in3), bap=bap,
                ftmm=cols(ftmm), ftb2=ftb2, gwbd=gwbd,
                ga1=ga1, ga2=ga2, ghbd=ghbd, gbc=gbc, cnrow4=cnrow4,
                onesrow=np.ones((1, 4224), f32))


CONST_SHAPES = dict(w0bd=(110, 3 * 128), i0=(128, 128),
                    wnodc=(64, 27 * 64),
                    in3=(128, 3 * 128), bap=(4, 3 * 128),
                    ftmm=(128, 2 * 128), ftb2=(128, 1), gwbd=(128, 128),
                    ga1=(128, 8), ga2=(128, 8), ghbd=(8, 2), gbc=(2, 128),
                    cnrow4=(2, 4), onesrow=(1, 4224))
# consts that feed float32r matmuls
FPR_CONSTS = {"i0", "wnodc", "in3", "bap", "gbc"}
BF_CONSTS = {"w0bd", "onesrow"}
# consts DMA'd from the scalar queue, rest from sync
SCALAR_Q_CONSTS = {"in3", "ftmm"}


def _cdtype(k):
    return BF if k in BF_CONSTS else (FPR if k in FPR_CONSTS else FP)


def _act_set_id():
    global ACT_SET_NLE
    if ACT_SET_NLE is None:
        from concourse.hw_specs import get_activation_tables
        for i, name in enumerate(get_activation_tables("gen3")):
            if name == "natural_log_exp_and_others":
                ACT_SET_NLE = i
                break
        assert ACT_SET_NLE is not None
    return ACT_SET_NLE


# ------------------------------------------------------------------ the module
def build_nc(nt=T, yw=(0.125, 0.5, 0.5, 0.5), yc=1.0):
    nc = bacc.Bacc(None, target_bir_lowering=False)
    xl1 = nc.declare_dram_parameter("xlin1", [XLEN], BF, isOutput=False)
    xl2 = nc.declare_dram_parameter("xlin2", [XLEN], BF, isOutput=False)
    cst = {k: nc.declare_dram_parameter(k, list(v), _cdtype(k),
                                        isOutput=False)
           for k, v in CONST_SHAPES.items()}
    y = nc.declare_dram_parameter("y", [T, BC, CO, HP, WP], BF, isOutput=True)

    with TileContext(nc) as tc:
        with (
            tc.tile_pool(name="consts", bufs=1) as cpool,
            tc.tile_pool(name="state", bufs=1) as spool,
            tc.tile_pool(name="im", bufs=1) as impool,
            tc.tile_pool(name="work", bufs=2) as wpool,
            tc.tile_pool(name="spc", bufs=3) as spc,
            tc.tile_pool(name="tiny", bufs=3) as tpool,
            tc.tile_pool(name="pconv", bufs=3, space="PSUM") as ps_conv,
            tc.tile_pool(name="pnode", bufs=3, space="PSUM") as ps_node,
            tc.tile_pool(name="ptiny", bufs=2, space="PSUM") as ps_tiny,
        ):
            # ---- preload the one ACT table set we use (Sign/Exp/Ln/Copy)
            ld = mybir.InstLoadActFuncSet(
                name=f"I-{nc.next_id()}", ins=[], outs=[],
                act_func_set_id=_act_set_id())
            nc.scalar.add_instruction(ld)

            # ---- im2col tiles + first two timesteps' taps FIRST so the
            # conv of block 0 is never waiting behind const DMAs
            imA = impool.tile([110, 4224], BF, tag="imA")
            imB = impool.tile([110, 4224], BF, tag="imB")

            def im2col(t):
                im = imA if t % 2 == 0 else imB
                i = 0
                for dy in range(3):
                    for base, xlin_ in ((2, xl1), (38, xl2), (74, xl1)):
                        p0 = base + dy * 12
                        eng = nc.scalar if i % 2 == 0 else nc.sync
                        i += 1
                        eng.dma_start(
                            im[p0:p0 + 12, :],
                            bass.AP(tensor=xlin_,
                                    offset=(128 + t * PLT
                                            + (dy - 1) * 66 - 1),
                                    ap=[[1, 3], [4224, 4], [1, 4224]]))

            im2col(0)
            im2col(1)

            # ---- im rows 0/1 (bias ones taps): DMA-broadcast from DRAM
            for imt in (imA, imB):
                nc.sync.dma_start(
                    imt[0:2, :],
                    bass.AP(tensor=cst["onesrow"], offset=0,
                            ap=[[0, 2], [1, 4224]]))

            # ---- consts to SBUF (split across both HWDGE queues)
            csb = {}
            for k, shp in CONST_SHAPES.items():
                if k == "onesrow":
                    continue
                t_ = cpool.tile(list(shp), _cdtype(k), tag=k)
                eng = nc.scalar if k in SCALAR_Q_CONSTS else nc.sync
                eng.dma_start(t_[:], cst[k][:])
                csb[k] = t_
            zcol = cpool.tile([128, 1], FP, tag="zcol")
            nc.vector.memset(zcol[:], 0.0)
            ocol = cpool.tile([128, 1], FP, tag="ocol")
            nc.vector.memset(ocol[:], 1.0)
            m4col = cpool.tile([128, 1], FP, tag="m4col")
            nc.vector.memset(m4col[:], -4.0)
            mcol = cpool.tile([128, 1], FP, tag="mcol")   # ACT bias -1
            nc.vector.memset(mcol[:], -1.0)
            actb = cpool.tile([128, 2], FP, tag="actb")
            nc.vector.memset(actb[:, 0:1], 0.0)
            nc.vector.memset(actb[:, 1:2], 1e-6)

            def bcfill(dst, src2d, *shape):
                nc.vector.tensor_copy(
                    dst, bass.AP(tensor=src2d.tensor, offset=src2d.offset,
                                 ap=[list(src2d.ap[0])]
                                 + [[0, s] for s in shape]))

            # ---- states (v0a filled FIRST — the t=0 conv needs it)
            v0a = spool.tile([128, 4096], FPR, tag="v0a")
            v0b = spool.tile([128, 4096], FPR, tag="v0b")
            vna = spool.tile([128, 3072], FPR, tag="vna")
            vnb = spool.tile([128, 3072], FPR, tag="vnb")
            Tt = spool.tile([128, 4], FP, tag="Tt")
            alprev = spool.tile([2, 4], FP, tag="alprev")
            in3s = spool.tile([128, 3 * 128], FPR, tag="in3s")
            bcfill(v0a[:], zcol[:, 0:1], 4096)
            nc.vector.memset(Tt[:], 0.0)
            nc.vector.memset(alprev[:], 1.0)

            # ---- persistent padded out0 tiles, pads preset to -4 once
            o0tiles = []
            for nm in ("o0A", "o0B"):
                o0t = spool.tile([128, 34 * 34], FPR, tag=nm)
                o0v = o0t[:].rearrange("p (h w) -> p h w", h=34)
                bcfill(o0v[:, 0, :], m4col[:, 0:1], 34)
                bcfill(o0v[:, 33, :], m4col[:, 0:1], 34)
                bcfill(o0v[:, 1:33, 0:1], m4col[:, 0:1], 32, 1)
                bcfill(o0v[:, 1:33, 33:34], m4col[:, 0:1], 32, 1)
                o0tiles.append(o0t)

            bcfill(vna[:], zcol[:, 0:1], 3072)
            # expand compact wnodc into the block-diag tap lhsT:
            # zero the tile, then two pattern-DMAs (one per batch half).
            wnod = cpool.tile([128, 27 * 128], FPR, tag="wnod")
            bcfill(wnod[:], zcol[:, 0:1], 27 * 128)
            wv = wnod[:].rearrange("p (k q) -> p k q", q=128)
            nc.sync.dma_start(wv[0:64, :, 0:64], csb["wnodc"][:])
            nc.scalar.dma_start(wv[64:128, :, 64:128], csb["wnodc"][:])
            csb["wnod"] = wnod

            # ---- bias rhs [4,4]: rows 0/1 get 1/a per (batch, node);
            # rows 2/3 = ones (static).
            bias_rhs = spool.tile([4, 4], FPR, tag="bias_rhs")
            bcfill(bias_rhs[:], ocol[0:4, 0:1], 4)



            def colmat(name, j):
                return csb[name][:, j * 128:(j + 1) * 128]
            ftb2ap = csb["ftb2"][:]

            def tiny(tag, p_, f_, dt_=FP):
                return tpool.tile([p_, f_], dt_, tag=tag, name=tag)

            def reap(ap_, tail):
                dims = [list(d) for d in ap_.ap][:-1] + tail
                return bass.AP(tensor=ap_.tensor, offset=ap_.offset,
                               ap=dims)

            def bc_n(ap_):  # [p,4] -> free (n,m): n varies, m bcast
                return reap(ap_, [[1, 4], [0, 4]])

            def bc_m(ap_):  # free (n,m): n bcast, m varies
                return reap(ap_, [[0, 4], [1, 4]])

            def tr_nm(ap_):  # read transposed over (n,m)
                return reap(ap_, [[1, 4], [4, 4]])

            def im2col(t):
                im = imA if t % 2 == 0 else imB
                i = 0
                for dy in range(3):
                    for base, xlin_ in ((2, xl1), (38, xl2), (74, xl1)):
                        p0 = base + dy * 12
                        eng = nc.scalar if i % 2 == 0 else nc.sync
                        i += 1
                        eng.dma_start(
                            im[p0:p0 + 12, :],
                            bass.AP(tensor=xlin_,
                                    offset=(128 + t * PLT
                                            + (dy - 1) * 66 - 1),
                                    ap=[[1, 3], [4224, 4], [1, 4224]]))

            # per-step tiles handed across blocks
            hand = {}

            # ---------------------------------------------- conv0 chunks
            def conv_chunk_fns(t):
                v0o, v0n = (v0a, v0b) if t % 2 == 0 else (v0b, v0a)
                im = imA if t % 2 == 0 else imB
                imv = im[:].rearrange("p (h w) -> p h w", h=64)
                p1 = wpool.tile([128, 32 * 64], BF, tag="p1", name="p1")
                f0acc = tiny("f0acc", 128, 8)
                o0r = o0tiles[t % 2][:].rearrange("p (h w) -> p h w", h=34)
                Wv = lambda v: csb["w0bd"][:, v * 128:(v + 1) * 128]

                def chunk(c):
                    sl = slice(c * 512, (c + 1) * 512)
                    ps = ps_conv.tile([128, 512], FP, tag="pc")
                    nc.tensor.matmul(ps[:], Wv(0),
                                     imv[:, c * 8:(c + 1) * 8, 0:64],
                                     start=True, stop=False)
                    if c == 0:
                        # subtract dy=0 taps' vertical-overflow garbage
                        nc.tensor.matmul(ps[:, 0:64], Wv(1),
                                         imv[:, 0:1, 0:64],
                                         start=False, stop=False,
                                         skip_group_check=True)
                    if c == 7:
                        nc.tensor.matmul(ps[:, 448:512], Wv(2),
                                         imv[:, 63:64, 0:64],
                                         start=False, stop=False,
                                         skip_group_check=True)
                    nc.tensor.matmul(ps[:], csb["i0"][:], v0o[:, sl],
                                     start=False, stop=True)
                    # s' = Sign(u-1) on ACT (+ f0 row-sum accum);
                    # v' = (u<1)*u on DVE, independent of the Sign
                    s0c = spc.tile([128, 512], BF, tag="s0c", name="s0c")
                    nc.scalar.activation(s0c[:], ps[:], Act.Sign,
                                         bias=mcol[:, 0:1], scale=1.0,
                                         accum_out=f0acc[:, c:c + 1])
                    nc.vector.scalar_tensor_tensor(
                        v0n[:, sl], s0c[:], 0.0, ps[:], Alu.is_lt, Alu.mult)
                    # vertical spike-pair sums on GPSIMD (keeps the DVE
                    # queue clear for resets + the serial graph chain)
                    s0r = s0c[:].rearrange("p (h w) -> p h w", h=8)
                    p1r = p1[:].rearrange("p (h w) -> p h w", h=32)
                    nc.gpsimd.tensor_tensor(
                        p1r[:, c * 4:(c + 1) * 4, :],
                        s0r[:, 0::2, :], s0r[:, 1::2, :], Alu.add)

                def assemble():
                    # horizontal pairs into padded out0 (GPSIMD)
                    p1r = p1[:].rearrange("p (h w) -> p h w", h=32)
                    nc.gpsimd.tensor_tensor(
                        o0r[:, 1:33, 1:33], p1r[:, :, 0::2],
                        p1r[:, :, 1::2], Alu.add)

                hand[("f0acc", t)] = f0acc
                return [lambda c=c: chunk(c) for c in range(8)], assemble

            # ---------------------------------------------- graph block
            def graph_segments(tg):
                seg = {}

                def g1():
                    f0acc = hand.pop(("f0acc", tg))
                    psf0 = ps_tiny.tile([128, 8], FP, tag="gt")
                    nc.tensor.matmul(psf0[:], colmat("ftmm", 0), f0acc[:],
                                     start=True, stop=True)
                    red8 = tiny("red8", 128, 1)
                    nc.vector.tensor_reduce(red8[:], psf0[:],
                                            mybir.AxisListType.X, Alu.add)
                    f04 = tiny("f04", 128, 1)
                    nc.vector.tensor_scalar(f04[:], red8[:], ftb2ap, 0.0,
                                            Alu.add, op1=Alu.max)
                    hand[("f04", tg)] = f04
                    nc.vector.scalar_tensor_tensor(
                        Tt[:, 0:1], Tt[:, 0:1], DECAY, f04[:],
                        Alu.mult, Alu.add)

                def g2():
                    psg = ps_tiny.tile([128, 4], FP, tag="gt")
                    nc.tensor.matmul(psg[:], csb["gwbd"][:], Tt[:],
                                     start=True, stop=True)
                    hpc = tiny("hpc", 128, 4)
                    nc.vector.tensor_copy(hpc[:], psg[:])
                    seg["hpc"] = hpc

                def g3():
                    hpc = seg["hpc"]
                    pse1 = ps_tiny.tile([8, 4], FP, tag="gt")
                    nc.tensor.matmul(pse1[:], csb["ga1"][:], hpc[:],
                                     start=True, stop=True)
                    pse2 = ps_tiny.tile([8, 4], FP, tag="gt")
                    nc.tensor.matmul(pse2[:], csb["ga2"][:], hpc[:],
                                     start=True, stop=True)
                    e1t = tiny("e1t", 8, 4)
                    nc.vector.tensor_copy(e1t[:], pse1[:])
                    es = tiny("es", 8, 16)
                    nc.vector.tensor_tensor(es[:], bc_n(e1t[:]),
                                            bc_m(pse2[:]), Alu.add)
                    el = tiny("el", 8, 16)
                    nc.vector.scalar_tensor_tensor(el[:], es[:], 0.2, es[:],
                                                   Alu.mult, Alu.max)
                    seg["el"] = el

                def g4():
                    el = seg["el"]
                    psE = ps_tiny.tile([2, 16], FP, tag="gt")
                    nc.tensor.matmul(psE[:], csb["ghbd"][:], el[:],
                                     start=True, stop=True)
                    Ec = tiny("Ec", 2, 16)
                    nc.vector.tensor_copy(Ec[:], psE[:])
                    L = tiny("L", 2, 16)
                    nc.vector.tensor_tensor(L[:], Ec[:], tr_nm(Ec[:]),
                                            Alu.add)
                    Lr = L[:].rearrange("p (n m) -> p n m", n=4)
                    mx = tiny("mx", 2, 4)
                    nc.vector.tensor_reduce(mx[:], Lr, mybir.AxisListType.X,
                                            Alu.max)
                    xm = tiny("xm", 2, 16)
                    nc.vector.tensor_tensor(xm[:], L[:], bc_n(mx[:]),
                                            Alu.subtract)
                    ex = tiny("ex", 2, 16)
                    nc.scalar.activation(ex[:], xm[:], Act.Exp,
                                         bias=actb[0:2, 0:1])
                    sm = tiny("sm", 2, 4)
                    exr = ex[:].rearrange("p (n m) -> p n m", n=4)
                    nc.vector.tensor_reduce(sm[:], exr, mybir.AxisListType.X,
                                            Alu.add)
                    rc = tiny("rc", 2, 4)
                    nc.vector.reciprocal(rc[:], sm[:])
                    S = tiny("S", 2, 16)
                    nc.vector.tensor_tensor(S[:], ex[:], bc_n(rc[:]),
                                            Alu.mult)

                    Sr = S[:].rearrange("p (n m) -> p n m", n=4)
                    lo = tiny("lo", 2, 8)
                    lor = lo[:].rearrange("p (n m) -> p n m", n=4)
                    hi = tiny("hi", 2, 8)
                    hir = hi[:].rearrange("p (n m) -> p n m", n=4)
                    nc.vector.tensor_tensor(lor, Sr[:, :, 0::2],
                                            Sr[:, :, 1::2], Alu.min)
                    nc.vector.tensor_tensor(hir, Sr[:, :, 0::2],
                                            Sr[:, :, 1::2], Alu.max)
                    kth = tiny("kth", 2, 4)
                    l2 = tiny("l2", 2, 4)
                    nc.vector.tensor_tensor(l2[:], lor[:, :, 0], lor[:, :, 1],
                                            Alu.max)
                    h2 = tiny("h2", 2, 4)
                    nc.vector.tensor_tensor(h2[:], hir[:, :, 0], hir[:, :, 1],
                                            Alu.min)
                    nc.vector.tensor_tensor(kth[:], l2[:], h2[:], Alu.min)
                    msk = tiny("msk", 2, 16)
                    nc.vector.tensor_tensor(msk[:], S[:], bc_n(kth[:]),
                                            Alu.is_ge)
                    Sp = tiny("Sp", 2, 16)
                    nc.vector.tensor_tensor(Sp[:], S[:], msk[:], Alu.mult)

                    A2 = tiny("A2", 2, 16)
                    nc.vector.tensor_tensor(A2[:], Sp[:], tr_nm(Sp[:]),
                                            Alu.add)
                    rs = tiny("rs", 2, 4)
                    A2r = A2[:].rearrange("p (n m) -> p n m", n=4)
                    nc.vector.tensor_reduce(rs[:], A2r, mybir.AxisListType.X,
                                            Alu.add)
                    lnd = tiny("lnd", 2, 4)
                    nc.scalar.activation(lnd[:], rs[:], Act.Ln,
                                         bias=actb[0:2, 1:2], scale=0.5)
                    q = tiny("q", 2, 4)
                    nc.scalar.activation(q[:], lnd[:], Act.Exp, scale=-0.5,
                                         bias=actb[0:2, 0:1])

                    t1 = tiny("t1", 2, 16)
                    nc.vector.tensor_tensor(t1[:], A2[:], bc_n(q[:]),
                                            Alu.mult)
                    OPt = tiny("OPt", 2, 16)
                    nc.vector.scalar_tensor_tensor(OPt[:], t1[:], 0.5,
                                                   bc_m(q[:]),
                                                   Alu.mult, Alu.mult)
                    col0 = reap(OPt[:], [[0, 4], [4, 4]])
                    t2 = tiny("t2", 2, 16)
                    nc.vector.tensor_tensor(t2[:], OPt[:], col0, Alu.mult)
                    af = tiny("af", 2, 4)
                    t2r = t2[:].rearrange("p (n m) -> p n m", n=4)
                    nc.vector.tensor_reduce(af[:], t2r, mybir.AxisListType.X,
                                            Alu.add)
                    # a = alpha*cn per (batch, node); clamp, invert, ratio
                    al3 = tiny("al3", 2, 4)
                    nc.vector.tensor_tensor(al3[:], af[:], csb["cnrow4"][:],
                                            Alu.mult)
                    am = tiny("am", 2, 4)
                    nc.vector.tensor_scalar(am[:], al3[:], 1e-30, None,
                                            Alu.max)
                    rc2 = tiny("rc2", 2, 4)
                    nc.vector.reciprocal(rc2[:], am[:])
                    X = tiny("X", 2, 8, FPR)
                    nc.vector.tensor_scalar(X[:, 0:4], rc2[:], -1.0,
                                            None, Alu.mult)
                    nc.vector.tensor_tensor(X[:, 4:8], alprev[:],
                                            rc2[:], Alu.mult)
                    nc.vector.tensor_copy(bias_rhs[0:2, :], rc2[:])
                    seg["X"] = X
                    seg["am"] = am

                def g5():
                    X, am = seg["X"], seg["am"]
                    psb = ps_tiny.tile([128, 8], FP, tag="gt")
                    nc.tensor.matmul(psb[:], csb["gbc"][:], X[:],
                                     start=True, stop=True)
                    aapX = tiny("aapX", 128, 8)
                    nc.vector.tensor_copy(aapX[:], psb[:])
                    # scaled identity blocks for the node LIF state add
                    for n in range(3):
                        nc.scalar.activation(
                            in3s[:, (n * 128):(n + 1) * 128],
                            csb["in3"][:, (n * 128):(n + 1) * 128],
                            Act.Copy, scale=aapX[:, 5 + n:6 + n])
                    nc.vector.tensor_copy(alprev[:], am[:])
                    hand[("aapX", tg)] = aapX

                seg["fns"] = (g1, g2, g3, g4, g5)
                return seg

            # ---------------------------------------------- node units
            def node_unit_fns(tp):
                vno, vnn = (vna, vnb) if tp % 2 == 0 else (vnb, vna)
                o0r = o0tiles[tp % 2][:].rearrange("p (h w) -> p h w", h=34)
                sn = wpool.tile([128, 3072], BF, tag="sn", name="sn")
                snsum = tiny("snsum", 128, 3)
                snsumB = tiny("snsumB", 128, 3)
                pstiles = {}

                def taps(u):
                    n, c = divmod(u, 2)
                    psn = ps_node.tile([128, 512], FP, tag="pn")
                    pstiles[u] = psn
                    for k in range(9):
                        dy, dx = k // 3, k % 3
                        rhs = o0r[:, dy + 16 * c: dy + 16 * c + 16,
                                  dx:dx + 32]
                        nc.tensor.matmul(
                            psn[:],
                            csb["wnod"][:, (n * 9 + k) * 128:
                                        (n * 9 + k + 1) * 128],
                            rhs, start=(k == 0), stop=False)

                def fin(u):
                    n, c = divmod(u, 2)
                    psn = pstiles.pop(u)
                    aapX = hand[("aapX", tp)]
                    nc.tensor.matmul(
                        psn[:], csb["bap"][:, n * 128:(n + 1) * 128],
                        reap(bias_rhs[:, n + 1:n + 2], [[0, 512]]),
                        start=False, stop=False, skip_group_check=True)
                    nc.tensor.matmul(
                        psn[:], in3s[:, n * 128:(n + 1) * 128],
                        vno[:, n * 1024 + c * 512:
                            n * 1024 + (c + 1) * 512],
                        start=False, stop=True)
                    sl = slice(n * 1024 + c * 512, n * 1024 + (c + 1) * 512)
                    nc.scalar.activation(
                        sn[:, sl], psn[:], Act.Sign,
                        bias=aapX[:, n + 1:n + 2],
                        accum_out=(snsum if c == 0 else snsumB)[:, n:n + 1])
                    nc.vector.scalar_tensor_tensor(
                        vnn[:, sl], sn[:, sl], 0.0, psn[:],
                        Alu.is_lt, Alu.mult)

                def ydma():
                    hand.pop(("aapX", tp))
                    # y as a sum of nonneg ZERO-EXACT terms (bf16-safe):
                    # y = yw0*(o0+4) + sum_n ywn*(sn'+1); no constant.
                    # All on GPSIMD (pure slack), incl. the SWDGE DMA.
                    ysb = wpool.tile([128, 1024], BF, tag="ysb", name="ysb")
                    nc.gpsimd.tensor_scalar(ysb[:], o0r[:, 1:33, 1:33],
                                            4.0, yw[0], Alu.add, op1=Alu.mult)
                    for n in range(3):
                        ytmp = wpool.tile([128, 1024], BF, tag="ytmp",
                                          name="ytmp")
                        nc.vector.tensor_scalar(
                            ytmp[:], sn[:, n * 1024:(n + 1) * 1024],
                            1.0, yw[n + 1], Alu.add, op1=Alu.mult)
                        nc.vector.tensor_tensor(ysb[:], ysb[:], ytmp[:],
                                                Alu.add)
                    nc.gpsimd.dma_start(
                        bass.AP(tensor=y, offset=tp * BC * CO * 1024,
                                ap=[[1024, 128], [1, 1024]]),
                        ysb[:])

                def ntail():
                    # feats + trace update (runs at the head of block tp+2)
                    f04 = hand.pop(("f04", tp))
                    psf = ps_tiny.tile([128, 3], FP, tag="gt")
                    nc.tensor.matmul(psf[:], colmat("ftmm", 1), snsum[:],
                                     start=True, stop=False)
                    nc.tensor.matmul(psf[:], colmat("ftmm", 1), snsumB[:],
                                     start=False, stop=True)
                    fn04 = tiny("fn04", 128, 3)
                    nc.vector.tensor_scalar(fn04[:], psf[:], ftb2ap, 0.0,
                                            Alu.add, op1=Alu.max)
                    nc.vector.scalar_tensor_tensor(
                        Tt[:, 0:1], Tt[:, 0:1], DECAY, f04[:], Alu.mult,
                        Alu.add)
                    nc.vector.scalar_tensor_tensor(
                        Tt[:, 1:4], Tt[:, 1:4], DECAY, fn04[:], Alu.mult,
                        Alu.add)

                return taps, fin, ydma, ntail

            # ---------------------------------------------- block schedule
            pending_ntail = [None, None]  # ntail closures for t-1, t-2
            tail_g5 = [None]

            for t in range(nt + 1):
                has_conv = t < nt
                has_prev = t >= 1
                cc, assemble = conv_chunk_fns(t) if has_conv else (None, None)
                if has_prev:
                    if t == nt:
                        # graph(nt-1) g1-g4 already ran inside block nt-1
                        g5 = tail_g5[0]
                        taps, fin, ydma, ntail = node_unit_fns(t - 1)
                        taps(0)
                        taps(1)
                        taps(2)
                        g5()
                        fin(0)
                        taps(3)
                        fin(1)
                        taps(4)
                        fin(2)
                        taps(5)
                        fin(3)
                        fin(4)
                        fin(5)
                        ydma()
                        break
                    gseg = graph_segments(t - 1)
                    g1, g2, g3, g4, g5 = gseg["fns"]
                    taps, fin, ydma, ntail = node_unit_fns(t - 1)

                cc[0]()
                if pending_ntail[0] is not None:
                    pending_ntail[0]()      # ntail(t-2)
                cc[1]()
                cc[2]()
                if has_prev:
                    g1()
                cc[3]()
                if has_prev:
                    g2()
                cc[4]()
                if has_prev:
                    g3()
                cc[5]()
                if has_prev:
                    g4()
                cc[6]()
                if has_prev:
                    taps(0)
                cc[7]()
                assemble()
                if t + 2 < nt:
                    im2col(t + 2)
                if has_prev:
                    taps(1)
                    g5()
                    taps(2)
                    fin(0)
                    taps(3)
                    fin(1)
                    taps(4)
                    fin(2)
                    taps(5)
                    fin(3)
                    fin(4)
                    fin(5)
                    if t == nt - 1:
                        # overlap the LAST step's graph chain with this
                        # block instead of exposing it in the tail
                        ntail()
                        lseg = graph_segments(t)
                        lg1, lg2, lg3, lg4, lg5 = lseg["fns"]
                        lg1()
                        lg2()
                        lg3()
                        lg4()
                        tail_g5[0] = lg5
                        pending_ntail = [None, None]
                    else:
                        pending_ntail = [ntail, None]
                    ydma()
                else:
                    pending_ntail = [None, None]
            # final trace updates are not needed (nothing consumes them)
    if not nc.is_finalized():
        nc.finalize()
    return nc


_NC_CACHE = {}


def _get_nc(nt=T, yw=(0.125, 0.5, 0.5, 0.5), yc=1.0):
    key = (nt, tuple(float(v) for v in yw), float(yc))
    if key not in _NC_CACHE:
        _NC_CACHE[key] = build_nc(nt, yw, yc)
    return _NC_CACHE[key]


def _prep_xlin(x_core):
    """x_core [T, BC, CIN, H, W] fp32 -> two flat padded BF16 arrays of
    length XLEN: bf16 hi part and bf16 residual, rows padded to 66 wide,
    plus 128-elem zero head/tail."""
    xb = np.ascontiguousarray(x_core, np.float32)
    hi = xb.astype(BF16)
    lo = (xb - hi.astype(np.float32)).astype(BF16)
    out = []
    for part in (hi, lo):
        pad = np.zeros(part.shape[:-1] + (66,), BF16)
        pad[..., :64] = part
        flat = np.zeros((XLEN,), BF16)
        flat[128:128 + T * PLT] = pad.reshape(-1)
        out.append(flat)
    return out


def kernel(**inputs):
    x = np.asarray(inputs["x"], np.float32)
    consts = _host_consts(
        inputs["conv0_w"], inputs["bn0_g"], inputs["bn0_b"], inputs["bn0_m"],
        inputs["bn0_v"], inputs["lif0_w"], inputs["convs_w"], inputs["bns_g"],
        inputs["bns_b"], inputs["bns_m"], inputs["bns_v"], inputs["lifs_w"],
        inputs["ft_w"], inputs["ft_b"], inputs["gat_w"], inputs["gat_a"],
        inputs["out_weights"])
    consts = {k: (np.ascontiguousarray(v, BF16) if k in BF_CONSTS
                  else np.ascontiguousarray(v, np.float32))
              for k, v in consts.items()}
    sigw = 1.0 / (1.0 + np.exp(-np.asarray(inputs["out_weights"], np.float64)))
    yw = (float(sigw[0]) / 8.0, float(sigw[1]) / 2.0, float(sigw[2]) / 2.0,
          float(sigw[3]) / 2.0)
    yc = float(sigw[0] / 2.0 + (sigw[1] + sigw[2] + sigw[3]) / 2.0)
    nc = _get_nc(T, yw, yc)
    core_ids = list(range(NCORES))
    in_maps = []
    for k in core_ids:
        m = dict(consts)
        x1, x2 = _prep_xlin(x[:, k * BC:(k + 1) * BC])
        m["xlin1"] = x1
        m["xlin2"] = x2
        in_maps.append(m)
    res = run_bass_kernel_spmd(nc, in_maps, core_ids).results
    out = np.concatenate([np.asarray(res[k]["y"]) for k in core_ids], axis=1)
    return out.astype(np.float32)
